# revision 14
# baseline (speedup 1.0000x reference)
"""Trainium2 Bass kernel for nn_Attention_Layer (B=8, SH=SV=32, DH=D=256, DV=4096).

Math (see reference):
    U_h = h @ U                  (B,SH,D)
    W_v = v @ W                  (B,SV,D)
    f   = tanh(W_v + U_h + b)    (B,SH,SV,D)
    q   = f @ w                  (B,SH,SV,DV)
    e   = exp(q); S = sum_b e; beta = e/S
    u   = sum_sv beta * v        (B,SH,DV)

Sharding: the batch-axis normalization (sum over b) makes batch sharding need a
16MB all-reduce; sharding over SH instead keeps everything core-local.
Each of the 8 cores owns SH/8 = 4 h-positions, all batches. No collectives.

Per-core design (v3):
  - All HBM tensors pre-laid host-side in SBUF layout; weight DMAs chunked so
    per-partition runs are >=2KB (efficient packets) and the Wv matmul starts
    on the first chunk.
  - W/vT stored fp8e4 (x32 scale on W, folded back via tanh scale=1/32):
    halves the prologue DMA.  Matmuls all run normal bf16/fp8 mode at
    1 col/cycle; DoubleRow measured useless on this silicon (streams at
    1.2GHz and does not keep the HAM clock-gate warm).
  - PE warmup matmuls + chunk-paced Wv keep HAM at K=8/8 from early on.
  - layout "fT": f^T stored (d, (b,h,s)); q matmul: fT stationary, w moving.
  - post-q "layout-1": partition=(h,s) [4*32=128], free=c' (DV), one tile per
    b.  S=sum_b e is a PE identity-matmul accumulation, beta=e*R needs no
    broadcast, u=sum_s (beta*v) is a PE matmul with per-b block-indicator
    stationary packed 4 concurrent col-groups.
  - v replicated over h in pair-sliced layout [128, NPAIR, B, NW2] via DMAs
    spread over the scalar/vector queues, pair-major so gv(pair0) unblocks
    early.
  - u-block for pair pr is emitted inside pair pr+1's PE stream; output is
    written bf16 and upcast on host.
"""

import sys

sys.path.insert(0, "/opt/trn_rl_repo")

from contextlib import ExitStack

import ml_dtypes
import numpy as np

import concourse.bass as bass
import concourse.mybir as mybir
import concourse.tile as tile
from concourse import bacc
from concourse.bass_utils import run_bass_kernel_spmd

BF16 = ml_dtypes.bfloat16
FP8 = ml_dtypes.float8_e4m3fn
F32 = np.float32

B, SH, SV, DH, DV, D = 8, 32, 32, 256, 4096, 256
NCORES = 8
SHL = SH // NCORES  # 4 h-positions per core
ROWS = B * SHL  # 32 output rows per core, index = b*SHL + h
NT = 8  # number of c' tiles
NW = DV // NT  # 512 wide
NPAIR = NT // 2  # q/exp run 1024-wide (two n-tiles at a time)
NW2 = 2 * NW  # 1024
KT_C = DV // 128  # 32 k-tiles over the DV contraction (v @ W)
KT_D = D // 128  # 2 k-tiles over the D contraction (f @ w)
NCHUNK = 2  # W/vT DMA chunks (16 k-tiles each -> 4KB/partition runs)
KPC = KT_C // NCHUNK
WSCALE = 32.0  # fp8 scale for W


def build_nc(debug: bool = False):
    nc = bacc.Bacc("TRN2", target_bir_lowering=False, debug=debug)
    f32, bf, f8 = mybir.dt.float32, mybir.dt.bfloat16, mybir.dt.float8e4

    # HBM tensors, all pre-laid in SBUF layout so each DMA below reads a
    # fully contiguous HBM range into [part x contiguous-run] SBUF.
    W8_d = nc.dram_tensor("W8", (NCHUNK, 128, KPC, D), f8, kind="ExternalInput")
    vT8_d = nc.dram_tensor(
        "vT8", (NCHUNK, 128, KPC, B * SV), f8, kind="ExternalInput"
    )
    w_d = nc.dram_tensor("wbf", (2, 128, KT_D, DV // 2), bf, kind="ExternalInput")
    v4_d = nc.dram_tensor("v4", (NPAIR, 128, B, NW2), bf, kind="ExternalInput")
    U2_d = nc.dram_tensor("U2", (128, 3, D), bf, kind="ExternalInput")
    hT2_d = nc.dram_tensor("hT2", (128, 3, ROWS), bf, kind="ExternalInput")
    I_d = nc.dram_tensor("Ieye", (128, 128), bf, kind="ExternalInput")
    L_d = nc.dram_tensor("Lsum", (128, B, ROWS), bf, kind="ExternalInput")
    u_d = nc.dram_tensor("u_out", (ROWS, DV), bf, kind="ExternalOutput")

    with tile.TileContext(nc) as tc, ExitStack() as ctx:
        consts = ctx.enter_context(tc.tile_pool(name="consts", bufs=1))

        ph1_ctx = ExitStack()
        ph1c = ph1_ctx.enter_context(tc.tile_pool(name="ph1c", bufs=1))

        # ---- DMA schedule -------------------------------------------------
        # sync:   I, U2, hT2, L, W8 chunks, w halves, v4 pair 2
        # gpsimd: vT8 chunks, v4 pairs 0/1/3, u_out writes later
        # scalar: no DMAs (keeps tanh/exp unblocked)
        I_sb = consts.tile([128, 128], bf)
        nc.sync.dma_start(out=I_sb, in_=I_d[:])
        U2_sb = ph1c.tile([128, 3, D], bf)
        nc.sync.dma_start(out=U2_sb, in_=U2_d[:])
        hT2_sb = ph1c.tile([128, 3, ROWS], bf)
        nc.sync.dma_start(out=hT2_sb, in_=hT2_d[:])
        L_sb = consts.tile([128, B, ROWS], bf)
        nc.sync.dma_start(out=L_sb, in_=L_d[:])

        # critical weights spread over all 3 DMA queues for parallel transfer
        W8_sb = ph1c.tile([128, KT_C, D], f8)
        vT8_sb = ph1c.tile([128, KT_C, B * SV], f8)
        ks0 = slice(0, KPC)
        ks1 = slice(KPC, 2 * KPC)
        nc.sync.dma_start(out=W8_sb[:, ks0, :], in_=W8_d[0])
        nc.gpsimd.dma_start(out=vT8_sb[:, ks0, :], in_=vT8_d[0])
        nc.scalar.dma_start(out=W8_sb[:, ks1, :], in_=W8_d[1])
        nc.gpsimd.dma_start(out=vT8_sb[:, ks1, :], in_=vT8_d[1])
        w_sb = consts.tile([128, KT_D, DV], bf)
        for wh in range(2):
            nc.sync.dma_start(
                out=w_sb[:, :, wh * (DV // 2) : (wh + 1) * (DV // 2)],
                in_=w_d[wh],
            )

        # v pre-replicated over the 4 local h positions host-side; one HBM
        # load per pair slab, pair-major so gv(pair0) unblocks early.
        v_rep = consts.tile([128, NPAIR, B, NW2], bf)
        nc.gpsimd.dma_start(out=v_rep[:, 0, :, :], in_=v4_d[0])
        nc.gpsimd.dma_start(out=v_rep[:, 1, :, :], in_=v4_d[1])
        nc.sync.dma_start(out=v_rep[:, 2, :, :], in_=v4_d[2])
        nc.gpsimd.dma_start(out=v_rep[:, 3, :, :], in_=v4_d[3])

        # ---- PE warmup + ACT table preload during the DMA prologue --------
        warm_pool = ph1_ctx.enter_context(
            tc.tile_pool(name="warm", bufs=1, space="PSUM")
        )
        warm_ps = warm_pool.tile([128, 128], f32)
        for it in range(24):
            nc.tensor.matmul(warm_ps, I_sb, I_sb, start=True, stop=True)
        warm_sb = ph1c.tile([128, 1], f32)
        nc.scalar.activation(
            warm_sb, warm_ps[:, 0:1], mybir.ActivationFunctionType.Exp
        )

        # ---- phase 1: ubias = 32*(U^T h + bias), W_v^T, fT = tanh(zz/32) --
        ph1 = ph1_ctx.enter_context(tc.tile_pool(name="ph1", bufs=1, space="PSUM"))

        ub_ps = ph1.tile([128, 2, ROWS], f32)
        for mt in range(2):
            msl = slice(mt * 128, (mt + 1) * 128)
            for kt in range(3):
                ksz = 128 if kt < 2 else 1
                nc.tensor.matmul(
                    ub_ps[:, mt, :],
                    U2_sb[0:ksz, kt, msl],
                    hT2_sb[0:ksz, kt, :],
                    start=(kt == 0),
                    stop=(kt == 2),
                )
        ub_sb = ph1c.tile([128, 2, ROWS], f32)
        nc.vector.tensor_copy(ub_sb, ub_ps)

        # Wv^T: 32 k-tiles x 2 mt, paced by the W8/vT8 chunk DMAs.
        wv_ps = [
            ph1.tile([128, B * SV], f32, tag=f"wv{mt}", name=f"wv_ps{mt}")
            for mt in range(2)
        ]
        for kt in range(KT_C):
            for mt in range(2):
                msl = slice(mt * 128, (mt + 1) * 128)
                nc.tensor.matmul(
                    wv_ps[mt],
                    W8_sb[:, kt, msl],
                    vT8_sb[:, kt, :],
                    start=(kt == 0),
                    stop=(kt == KT_C - 1),
                )

        # zz[d, (b,h,s)] = 32*(W_v^T + ubias); fT = tanh(zz/32) in bf16
        zz_sb = ph1c.tile([128, 2, B * SHL * SV], f32)
        fT_sb = consts.tile([128, KT_D, B * SHL * SV], bf)
        for mt in range(2):
            wv_base = wv_ps[mt][:]
            wv_bc = bass.AP(
                tensor=wv_base.tensor,
                offset=wv_base.offset,
                ap=[wv_base.ap[0], [32, B], [0, SHL], [1, SV]],
            )
            ub_base = ub_sb[:, mt, :]
            ub_bc = bass.AP(
                tensor=ub_base.tensor,
                offset=ub_base.offset,
                ap=[ub_base.ap[0], [SHL, B], [1, SHL], [0, SV]],
            )
            zz_out = zz_sb[:, mt, :].rearrange("p (b h s) -> p b h s", b=B, h=SHL)
            nc.vector.tensor_add(zz_out, wv_bc, ub_bc)
            for bh in range(2):  # split so the first q-matmuls start earlier
                bsl = slice(bh * 512, (bh + 1) * 512)
                nc.scalar.activation(
                    fT_sb[:, mt, bsl],
                    zz_sb[:, mt, bsl],
                    mybir.ActivationFunctionType.Tanh,
                    scale=1.0 / WSCALE,
                )

        ph1_ctx.close()

        # ---- phase 2: q -> e -> S -> R -> beta*v -> u ---------------------
        epool = ctx.enter_context(tc.tile_pool(name="epool", bufs=2))
        gpool = ctx.enter_context(tc.tile_pool(name="gpool", bufs=4))
        gvpool = ctx.enter_context(tc.tile_pool(name="gvpool", bufs=16))
        usbpool = ctx.enter_context(tc.tile_pool(name="usbpool", bufs=3))
        r32pool = ctx.enter_context(tc.tile_pool(name="r32pool", bufs=2))
        rpool = ctx.enter_context(tc.tile_pool(name="rpool", bufs=2))
        qpool = ctx.enter_context(tc.tile_pool(name="qpool", bufs=2, space="PSUM"))
        spool = ctx.enter_context(tc.tile_pool(name="spool", bufs=2, space="PSUM"))
        upool = ctx.enter_context(tc.tile_pool(name="upool", bufs=2, space="PSUM"))

        NJ = 4  # col-group packing of the M=32 u-matmuls: 4 concurrent MMs
        NWJ = NW2 // NJ  # 256

        def emit_u_block(gv_tiles, pr):
            # 4 col-groups, each contracting a different 256-wide c' chunk;
            # rows (b,h) land at partition offset 32*j for chunk j.
            u_ps = upool.tile([128, NWJ], f32)
            for bb in range(B):
                for j in range(NJ):
                    nc.tensor.matmul(
                        u_ps[32 * j : 32 * (j + 1), :],
                        L_sb[:, bb, :],
                        gv_tiles[bb][:, j * NWJ : (j + 1) * NWJ],
                        start=(bb == 0),
                        stop=(bb == B - 1),
                        tile_position=(0, 32 * j),
                        skip_group_check=True,
                    )
            u_sb = usbpool.tile([ROWS, NW2], bf, tag="u_sb")
            for j in range(NJ):
                src = u_ps[32 * j : 32 * (j + 1), :]
                dst = u_sb[:, j * NWJ : (j + 1) * NWJ]
                if j % 2 == 0:
                    nc.scalar.copy(dst, src)
                else:
                    nc.vector.tensor_copy(dst, src)
            nc.gpsimd.dma_start(out=u_d[:, pr * NW2 : (pr + 1) * NW2], in_=u_sb)

        def emit_u_half(gv_tiles, nt):
            # 512-wide variant for the final tiles; copies split ACT/DVE so
            # the post-last-matmul serial chain is shorter.
            nwj = NW // NJ  # 128
            u_ps = upool.tile([128, nwj], f32, tag="u_ps", name=f"u_ps_h{nt}")
            for bb in range(B):
                for j in range(NJ):
                    nc.tensor.matmul(
                        u_ps[32 * j : 32 * (j + 1), :],
                        L_sb[:, bb, :],
                        gv_tiles[bb][:, j * nwj : (j + 1) * nwj],
                        start=(bb == 0),
                        stop=(bb == B - 1),
                        tile_position=(0, 32 * j),
                        skip_group_check=True,
                    )
            u_sb = usbpool.tile([ROWS, NW], bf, tag="u_sb", name=f"u_sb_h{nt}")
            for j in range(NJ):
                src = u_ps[32 * j : 32 * (j + 1), :]
                dst = u_sb[:, j * nwj : (j + 1) * nwj]
                if j % 2 == 0:
                    nc.scalar.copy(dst, src)
                else:
                    nc.vector.tensor_copy(dst, src)
            nc.gpsimd.dma_start(out=u_d[:, nt * NW : (nt + 1) * NW], in_=u_sb)

        pending = None  # u-block deferred by one pair

        for pr in range(NPAIR):
            last = pr == NPAIR - 1

            # q matmuls (fT stationary, kt-outer so stationary covers both
            # halves) + exp
            e_all = epool.tile([128, B, NW2], bf, tag="e", name=f"e_{pr}")
            for bb in range(B):
                bsl = slice(bb * 128, (bb + 1) * 128)
                q_ps = qpool.tile([128, NW2], f32)
                for kt in range(KT_D):
                    for half in range(2):
                        nt = 2 * pr + half
                        nsl = slice(nt * NW, (nt + 1) * NW)
                        nc.tensor.matmul(
                            q_ps[:, half * NW : (half + 1) * NW],
                            fT_sb[:, kt, bsl],
                            w_sb[:, kt, nsl],
                            start=(kt == 0),
                            stop=(kt == KT_D - 1),
                            skip_group_check=True,
                        )
                nc.scalar.activation(
                    e_all[:, bb, :],
                    q_ps,
                    mybir.ActivationFunctionType.Exp,
                )

            # previous pair's u-block rides here: fills the PE gap while the
            # last exps of this pair finish.
            if pending is not None:
                emit_u_block(*pending)
                pending = None

            # S = sum_b e via identity-matmul accumulation, then R = 1/S
            # (reciprocal reads the PSUM accumulator directly)
            r_pair = rpool.tile([128, 2, NW], bf, tag="r")
            r32p = r32pool.tile([128, 2, NW], f32, tag="r32")
            for half in range(2):
                s_ps = spool.tile([128, NW], f32)
                for bb in range(B):
                    nc.tensor.matmul(
                        s_ps,
                        I_sb,
                        e_all[:, bb, half * NW : (half + 1) * NW],
                        start=(bb == 0),
                        stop=(bb == B - 1),
                    )
                nc.vector.reciprocal_approx_fast(r32p[:, half, :], s_ps)
                if last:
                    # per-half chain: r needed immediately for the tail g/gv
                    nc.scalar.copy(r_pair[:, half, :], r32p[:, half, :])
                    nt = 2 * pr + half
                    gv_half = []
                    for bb in range(B):
                        g_t = gpool.tile([128, NW], bf, tag="g")
                        nc.vector.tensor_mul(
                            g_t,
                            e_all[:, bb, half * NW : (half + 1) * NW],
                            r_pair[:, half, :],
                        )
                        gv_t = gvpool.tile([128, NW], bf, tag="gv")
                        nc.vector.tensor_mul(
                            gv_t,
                            g_t,
                            v_rep[:, pr, bb, half * NW : (half + 1) * NW],
                        )
                        gv_half.append(gv_t)
                    emit_u_half(gv_half, nt)

            if not last:
                # pair-wide cast, then per-b g and gv multiplies
                nc.scalar.copy(
                    r_pair.rearrange("p a n -> p (a n)"),
                    r32p.rearrange("p a n -> p (a n)"),
                )
                gv_tiles = []
                for bb in range(B):
                    g_t = gpool.tile([128, NW2], bf, tag="g")
                    nc.vector.tensor_mul(
                        g_t,
                        e_all[:, bb, :],
                        r_pair.rearrange("p a n -> p (a n)"),
                    )
                    gv_t = gvpool.tile([128, NW2], bf, tag="gv")
                    nc.vector.tensor_mul(gv_t, g_t, v_rep[:, pr, bb, :])
                    gv_tiles.append(gv_t)
                pending = (gv_tiles, pr)

    nc.compile()
    return nc


def _install_profile_hook():
    """The image's antenv lacks axon_hooks; inject it and register the
    ctypes NTFF hook from trn_agent_boot so trace=True works under axon."""
    import types

    try:
        from antenv.axon_hooks import get_axon_ntff_profile_hook  # noqa: F401

        return
    except ImportError:
        pass
    import antenv

    mod = types.ModuleType("antenv.axon_hooks")
    holder = {"hook": None}
    mod.set_axon_ntff_profile_hook = lambda h: holder.__setitem__("hook", h)
    mod.get_axon_ntff_profile_hook = lambda: holder["hook"]
    sys.modules["antenv.axon_hooks"] = mod
    antenv.axon_hooks = mod
    try:
        if "/root/.axon_site" not in sys.path:
            sys.path.insert(0, "/root/.axon_site")
        from trn_agent_boot.trn_boot import _ntff_profile_via_ctypes

        mod.set_axon_ntff_profile_hook(
            _ntff_profile_via_ctypes("/opt/axon/libaxon_pjrt.so")
        )
    except Exception as ex:  # degrade: tracing skipped, run still works
        print("profile hook install failed:", ex)
    # artifact upload needs bucket creds this container doesn't have
    import concourse.bass_utils as bu

    bu.upload_artifacts = lambda tmpdir: "local://" + tmpdir


_NC_CACHE = {}


def _get_nc():
    if "nc" not in _NC_CACHE:
        _NC_CACHE["nc"] = build_nc()
    return _NC_CACHE["nc"]


def make_inputs(h, v, W, U, b, w):
    """Host-side prep: shared tensors + per-core in_maps.  Every HBM tensor
    is pre-arranged into its SBUF layout so DMAs are plain 2D copies."""
    # W8: (NCHUNK, 128, KPC, D) from (KT_C, 128, D) k-tile layout
    W8 = np.ascontiguousarray(
        (W * WSCALE).reshape(NCHUNK, KPC, 128, D).transpose(0, 2, 1, 3)
    ).astype(FP8)
    vT = np.ascontiguousarray(v.transpose(2, 0, 1).reshape(DV, B * SV))
    vT8 = np.ascontiguousarray(
        vT.reshape(NCHUNK, KPC, 128, B * SV).transpose(0, 2, 1, 3)
    ).astype(FP8)
    # wbf: (2, 128, KT_D, DV//2)
    wbf = np.ascontiguousarray(
        w.reshape(KT_D, 128, 2, DV // 2).transpose(2, 1, 0, 3)
    ).astype(BF16)
    # v4: (NPAIR, 128=(h,s), B, NW2), v replicated over the 4 h positions
    vv = v.reshape(B, SV, NPAIR, NW2).transpose(2, 1, 0, 3)  # (pr, s, b, c)
    v4 = np.ascontiguousarray(np.tile(vv, (1, SHL, 1, 1))).astype(BF16)
    U2 = np.concatenate([U * WSCALE, b[None, :] * WSCALE], axis=0).astype(F32)
    U2p = np.zeros((128, 3, D), dtype=BF16)
    U2p[:, 0, :] = U2[0:128]
    U2p[:, 1, :] = U2[128:256]
    U2p[0, 2, :] = U2[256]
    Ieye = np.eye(128, dtype=BF16)
    Lsum = np.zeros((128, B, ROWS), dtype=BF16)
    for bb in range(B):
        for hh in range(SHL):
            for ss in range(SV):
                Lsum[hh * SV + ss, bb, bb * SHL + hh] = 1
    in_maps = []
    for core in range(NCORES):
        hsl = h[:, core * SHL : (core + 1) * SHL, :]  # (B, SHL, DH)
        hT = np.ascontiguousarray(hsl.transpose(2, 0, 1).reshape(DH, ROWS))
        hT2p = np.zeros((128, 3, ROWS), dtype=BF16)
        hT2p[:, 0, :] = hT[0:128]
        hT2p[:, 1, :] = hT[128:256]
        hT2p[0, 2, :] = 1.0
        in_maps.append(
            {
                "W8": W8,
                "vT8": vT8,
                "wbf": wbf,
                "v4": v4,
                "U2": U2p,
                "hT2": hT2p,
                "Ieye": Ieye,
                "Lsum": Lsum,
            }
        )
    return in_maps


def gather_output(results):
    u_full = np.empty((B, SH, DV), dtype=F32)
    for core, res in enumerate(results):
        u_full[:, core * SHL : (core + 1) * SHL, :] = (
            res["u_out"].astype(F32).reshape(B, SHL, DV)
        )
    return u_full


def kernel(h, v, W, U, b, w, trace: bool = False):
    if trace:
        _install_profile_hook()
    nc = _get_nc()
    in_maps = make_inputs(
        np.asarray(h, F32),
        np.asarray(v, F32),
        np.asarray(W, F32),
        np.asarray(U, F32),
        np.asarray(b, F32),
        np.asarray(w, F32),
    )
    out = run_bass_kernel_spmd(nc, in_maps, core_ids=list(range(NCORES)), trace=trace)
    res = gather_output(out.results)
    if trace:
        kernel.last_exec_time_ns = out.exec_time_ns
        kernel.last_trace = out.instructions_and_trace
    return res
